# revision 3
# baseline (speedup 1.0000x reference)
"""ECPGLinear (ternary-quantized linear) Bass kernel for 8 TRN2 NeuronCores.

Computes out = x @ W.T where W = dequant(ternary, per-group scales),
group_size=128 along in_features — a 8192x4096x4096 matmul. Data-parallel
over the 8192 tokens: each core takes 1024 rows of x and the full weight
matrix; no collectives, the host concatenates the 8 output shards.

Hybrid-precision schedule (fp32 PSUM accumulate):
  - k-tiles 0..7 (first 1024 in_features) run as 4 fp8e4m3 DoubleRow
    pairs: both x and dequantized W quantized to e4m3 on the host; each
    DoubleRow matmul contracts 256 rows in the time of one fp16 matmul
    (2x PE throughput). Measured end-to-end rel err 1.9e-2 vs the 2e-2
    budget (e4m3 carries ~2.7% RMS per operand; 8/32 of the contraction
    at 3.8% -> sqrt(8/32)*3.8% = 1.9%).
  - k-tiles 8..31 run in fp16 (near-exact, ~3.6e-4).
  - Dequantization (ternary * group scale) happens ON THE HOST: the
    device streams pre-dequantized fp16/fp8 weight tiles straight into
    matmuls. This removes the on-device DVE dequant and the replicated
    per-partition scales DMA of the all-fp16 predecessor kernel.
  - Per-core PE work: 8 chunks x (24 fp16 + 4 DR) x 8 bank-tiles.
    Stationary = W subtile ([128k x 128o] fp16 or [128k x 2 x 128o]
    fp8), moving = resident x ([128k x 512m] fp16 or [128k x 2 x 512m]
    fp8), PSUM [128o x 512m]; 4 o-subtiles x 2 m-halves accumulate in 8
    PSUM banks across the 28 steps of each 512-col output chunk. Output
    lands transposed [OUT_F, M_CORE]; the host untransposes.
  - ACT evicts PSUM to SBUF as fp16; stores ride the gpsimd queue.
  - 13 warmup matmuls on memset tiles cover the PE clock ramp plus the
    first tiles' DMA+sem latency.
  - Tail: the last chunk runs as two 4-bank phases (m-half 0 then 1)
    reusing resident weight tiles, so phase A's evictions and stores
    overlap phase B's matmuls.

Host prep: dequantize W once in f32, then cast/layout shards (fp16 for
k-tiles 8..31, e4m3 for 0..7). Since ternary is in {-1,0,1}, host
rounding of w = t*s to fp16/e4m3 is exactly the quantized weight the
device would produce.
"""
import functools
import numpy as np

OUT_F = 4096
IN_F = 4096
B, S = 4, 2048
M_TOT = B * S
NCORES = 8
M_CORE = M_TOT // NCORES
KT = IN_F // 128
NP8 = 4                 # fp8 DoubleRow k-pairs (2 k-tiles each)
KT16 = KT - 2 * NP8     # fp16 k-tiles
NCH = OUT_F // 512
NWARM = 13
PREF = 5
STEPS = NP8 + KT16      # per-chunk producer/consumer steps
TOT = NCH * STEPS


@functools.lru_cache(maxsize=1)
def _build():
    from concourse import bacc
    import concourse.mybir as mybir
    import concourse.tile as tile

    f32 = mybir.dt.float32
    f16 = mybir.dt.float16
    f8 = mybir.dt.float8e4
    DR = mybir.MatmulPerfMode.DoubleRow

    nc = bacc.Bacc("TRN2", target_bir_lowering=False, debug=False,
                   num_devices=NCORES)
    # x shards, transposed: [in_features, m]
    x8t = nc.dram_tensor("x8t", [128, NP8, 2, M_CORE], f8,
                         kind="ExternalInput")
    x16t = nc.dram_tensor("x16t", [128, KT16, M_CORE], f16,
                          kind="ExternalInput")
    # pre-dequantized weights, transposed: [in_features, out_features]
    w8 = nc.dram_tensor("w8", [128, NP8, 2, OUT_F], f8,
                        kind="ExternalInput")
    w16 = nc.dram_tensor("w16", [KT16 * 128, OUT_F], f16,
                         kind="ExternalInput")

    # transposed output: [OUT_F, M_CORE]
    out = nc.dram_tensor("out", [OUT_F, M_CORE], f16, kind="ExternalOutput")

    with tile.TileContext(nc) as tc:
        with (
            tc.tile_pool(name="xres", bufs=1) as xres_pool,
            tc.tile_pool(name="wst", bufs=10) as wst_pool,
            tc.tile_pool(name="wdl", bufs=STEPS) as wdl_pool,
            tc.tile_pool(name="ost", bufs=12) as ost_pool,
            tc.tile_pool(name="psum", bufs=8, space="PSUM") as psum_pool,
        ):
            x8res = xres_pool.tile([128, NP8, 2, M_CORE], f8)
            x16res = xres_pool.tile([128, KT16, M_CORE], f16)

            warm_l = wst_pool.tile([128, 128], f16, name="warm_l", tag="warm")
            warm_r = xres_pool.tile([128, 512], f16, name="warm_r")
            nc.vector.memset(warm_l[:], 0.0)
            nc.vector.memset(warm_r[:], 0.0)
            warm_ps = psum_pool.tile([128, 512], f32, name="warm_ps",
                                     tag="ps")
            for _ in range(NWARM):
                nc.tensor.matmul(warm_ps[:], warm_l[:], warm_r[:],
                                 start=True, stop=True)

            wts = {}

            def produce(j):
                n, st = divmod(j, STEPS)
                o0 = n * 512
                pool = wdl_pool if n == NCH - 1 else wst_pool
                tg = "wdl" if pool is wdl_pool else "wst"
                if st < NP8:
                    a = st
                    if n == 0:
                        nc.sync.dma_start(x8res[:, a, :, :],
                                          x8t[:, a, :, :])
                    wt = pool.tile([128, 2, 512], f8,
                                   name=f"w8_{n}_{a}", tag=tg)
                    nc.gpsimd.dma_start(wt[:], w8[:, a, :, o0:o0 + 512])
                else:
                    kt = st - NP8
                    if n == 0:
                        nc.sync.dma_start(x16res[:, kt, :],
                                          x16t[:, kt, :])
                    wt = pool.tile([128, 512], f16,
                                   name=f"w16_{n}_{kt}", tag=tg)
                    nc.gpsimd.dma_start(
                        wt[:], w16[kt * 128:(kt + 1) * 128, o0:o0 + 512])
                wts[j] = wt

            psums = None

            def consume(j):
                nonlocal psums
                n, st = divmod(j, STEPS)
                last = n == NCH - 1
                mhs = (0,) if last else (0, 1)
                if st == 0:
                    psums = [psum_pool.tile([128, 512], f32,
                                            name=f"ps{n}_{o}_{mh}",
                                            tag="ps")
                             for o in range(4) for mh in mhs]
                wt = wts.pop(j) if not last else wts[j]
                for o in range(4):
                    for mh in mhs:
                        ps = psums[o * len(mhs) + mh]
                        if st < NP8:
                            nc.tensor.matmul(
                                ps[:],
                                wt[:, :, o * 128:(o + 1) * 128],
                                x8res[:, st, :, mh * 512:(mh + 1) * 512],
                                start=(st == 0),
                                stop=(st == STEPS - 1),
                                perf_mode=DR,
                            )
                        else:
                            nc.tensor.matmul(
                                ps[:],
                                wt[:, o * 128:(o + 1) * 128],
                                x16res[:, st - NP8,
                                       mh * 512:(mh + 1) * 512],
                                start=(st == 0),
                                stop=(st == STEPS - 1),
                            )
                if st == STEPS - 1 and not last:
                    o0 = n * 512
                    for o in range(4):
                        for mh in (0, 1):
                            ost = ost_pool.tile([128, 512], f16,
                                                name=f"ost{n}_{o}_{mh}",
                                                tag="ost")
                            nc.scalar.copy(ost[:], psums[o * 2 + mh][:])
                            nc.gpsimd.dma_start(
                                out[o0 + o * 128:o0 + (o + 1) * 128,
                                    mh * 512:(mh + 1) * 512],
                                ost[:])

            for j in range(TOT + PREF):
                if j < TOT:
                    produce(j)
                jc = j - PREF
                if jc >= 0:
                    consume(jc)

            # Last chunk finale: phase A was mh=0 (m 0..511), step-major
            # (tiles stream in); its 4 evictions overlap phase B. Phase B
            # (mh=1) runs BANK-major on the resident weight tiles: each
            # bank finishes its full 28-step sweep, then its eviction and
            # store overlap the next bank's matmuls — only the last
            # bank's drain is exposed at the end.
            n = NCH - 1
            o0 = n * 512
            psA = psums
            for o in range(4):
                ost = ost_pool.tile([128, 512], f16,
                                    name=f"ost{n}_{o}_0", tag="ost")
                if o % 2 == 0:
                    nc.vector.tensor_copy(ost[:], psA[o][:])
                else:
                    nc.scalar.copy(ost[:], psA[o][:])
                nc.gpsimd.dma_start(
                    out[o0 + o * 128:o0 + (o + 1) * 128, 0:512], ost[:])
            for o in range(4):
                psB = psum_pool.tile([128, 512], f32,
                                     name=f"ps{n}_{o}_1", tag="ps")
                for st in range(STEPS):
                    wt = wts[n * STEPS + st]
                    if st < NP8:
                        nc.tensor.matmul(
                            psB[:],
                            wt[:, :, o * 128:(o + 1) * 128],
                            x8res[:, st, :, 512:1024],
                            start=(st == 0),
                            stop=(st == STEPS - 1),
                            perf_mode=DR,
                        )
                    else:
                        nc.tensor.matmul(
                            psB[:],
                            wt[:, o * 128:(o + 1) * 128],
                            x16res[:, st - NP8, 512:1024],
                            start=(st == 0),
                            stop=(st == STEPS - 1),
                        )
                if o < 3:
                    ost = ost_pool.tile([128, 512], f16,
                                        name=f"ost{n}_{o}_1", tag="ost")
                    if o % 2 == 0:
                        nc.vector.tensor_copy(ost[:], psB[:])
                    else:
                        nc.scalar.copy(ost[:], psB[:])
                    dma = nc.sync if o % 2 == 0 else nc.scalar
                    dma.dma_start(
                        out[o0 + o * 128:o0 + (o + 1) * 128, 512:1024],
                        ost[:])
                else:
                    # final bank: split the drain across DVE+ACT and two
                    # HWDGE queues to halve the exposed tail
                    for hh in (0, 1):
                        ost = ost_pool.tile([128, 256], f16,
                                            name=f"ost{n}_{o}_1_{hh}",
                                            tag="ost")
                        if hh == 0:
                            nc.vector.tensor_copy(
                                ost[:], psB[:, 0:256])
                        else:
                            nc.scalar.copy(ost[:], psB[:, 256:512])
                        dma = nc.sync if hh == 0 else nc.scalar
                        dma.dma_start(
                            out[o0 + o * 128:o0 + (o + 1) * 128,
                                512 + hh * 256:512 + (hh + 1) * 256],
                            ost[:])

    nc.compile()
    return nc


def kernel(x: np.ndarray, ternary: np.ndarray, scales: np.ndarray,
           _trace: bool = False):
    import ml_dtypes
    from concourse.bass_utils import run_bass_kernel_spmd

    nc = _build()
    f8 = ml_dtypes.float8_e4m3

    x = np.asarray(x)
    ternary = np.asarray(ternary)
    scales = np.asarray(scales)

    # Dequantize on the host: W[o, i] = ternary[o, i] * scales[o, i//128]
    w = (ternary.astype(np.float32).reshape(-1, 128)
         * np.asarray(scales, dtype=np.float32)[:, None]).reshape(OUT_F, IN_F)
    wT = np.ascontiguousarray(w.T)  # [in, out]
    K8 = 2 * NP8 * 128  # in_features handled in fp8
    # [128, NP8, 2, OUT_F]: w8h[k, a, j, o] = Q8(wT[(2a+j)*128 + k, o])
    w8h = np.ascontiguousarray(
        wT[:K8].reshape(NP8, 2, 128, OUT_F).transpose(2, 0, 1, 3)
    ).astype(f8)
    w16h = wT[K8:].astype(np.float16)

    xf = x.reshape(M_TOT, IN_F)
    in_maps = []
    for c in range(NCORES):
        xcT = xf[c * M_CORE:(c + 1) * M_CORE, :].T  # [in, m]
        x8h = np.ascontiguousarray(
            xcT[:K8].reshape(NP8, 2, 128, M_CORE).transpose(2, 0, 1, 3)
        ).astype(f8)
        x16h = np.ascontiguousarray(
            xcT[K8:].reshape(KT16, 128, M_CORE).transpose(1, 0, 2)
        ).astype(np.float16)
        in_maps.append({"x8t": x8h, "x16t": x16h, "w8": w8h, "w16": w16h})

    res = run_bass_kernel_spmd(nc, in_maps, list(range(NCORES)),
                               trace=_trace)
    # out is [OUT_F, M_CORE] per core; untranspose on the host
    outs = [res.results[c]["out"].T for c in range(NCORES)]
    full = np.concatenate(outs, axis=0).astype(np.float32).reshape(B, S, OUT_F)
    if _trace:
        kernel.last_results = res
    return full


kernel.last_results = None


# revision 5
# speedup vs baseline: 1.0231x; 1.0231x over previous
"""ECPGLinear (ternary-quantized linear) Bass kernel for 8 TRN2 NeuronCores.

Computes out = x @ W.T where W = dequant(ternary, per-group scales),
group_size=128 along in_features — a 8192x4096x4096 matmul. Data-parallel
over the 8192 tokens: each core takes 1024 rows of x and the full weight
matrix; no collectives, the host concatenates the 8 output shards.

Hybrid-precision schedule (fp32 PSUM accumulate):
  - k-tiles 0..7 (first 1024 in_features) run as 4 fp8e4m3 DoubleRow
    pairs: both x and dequantized W quantized to e4m3 on the host; each
    DoubleRow matmul contracts 256 rows in the time of one fp16 matmul
    (2x PE throughput). Measured end-to-end rel err 1.9e-2 vs the 2e-2
    budget (e4m3 carries ~2.7% RMS per operand; 8/32 of the contraction
    at 3.8% -> sqrt(8/32)*3.8% = 1.9%).
  - k-tiles 8..31 run in fp16 (near-exact, ~3.6e-4).
  - Dequantization (ternary * group scale) happens ON THE HOST: the
    device streams pre-dequantized fp16/fp8 weight tiles straight into
    matmuls. This removes the on-device DVE dequant and the replicated
    per-partition scales DMA of the all-fp16 predecessor kernel.
  - Per-core PE work: 8 chunks x (24 fp16 + 4 DR) x 8 bank-tiles.
    Stationary = W subtile ([128k x 128o] fp16 or [128k x 2 x 128o]
    fp8), moving = resident x ([128k x 512m] fp16 or [128k x 2 x 512m]
    fp8), PSUM [128o x 512m]; 4 o-subtiles x 2 m-halves accumulate in 8
    PSUM banks across the 28 steps of each 512-col output chunk. Output
    lands transposed [OUT_F, M_CORE]; the host untransposes.
  - ACT evicts PSUM to SBUF as fp16; stores ride the gpsimd queue.
  - 13 warmup matmuls on memset tiles cover the PE clock ramp plus the
    first tiles' DMA+sem latency.
  - Tail: the last chunk runs as two 4-bank phases (m-half 0 then 1)
    reusing resident weight tiles, so phase A's evictions and stores
    overlap phase B's matmuls.

Host prep: dequantize W once in f32, then cast/layout shards (fp16 for
k-tiles 8..31, e4m3 for 0..7). Since ternary is in {-1,0,1}, host
rounding of w = t*s to fp16/e4m3 is exactly the quantized weight the
device would produce.
"""
import functools
import numpy as np

OUT_F = 4096
IN_F = 4096
B, S = 4, 2048
M_TOT = B * S
NCORES = 8
M_CORE = M_TOT // NCORES
KT = IN_F // 128
NP8 = 4                 # fp8 DoubleRow k-pairs (2 k-tiles each)
KT16 = KT - 2 * NP8     # fp16 k-tiles
NCH = OUT_F // 512
NWARM = 13
PREF = 5
STEPS = NP8 + KT16      # per-chunk producer/consumer steps
TOT = NCH * STEPS


@functools.lru_cache(maxsize=1)
def _build():
    from concourse import bacc
    import concourse.mybir as mybir
    import concourse.tile as tile

    f32 = mybir.dt.float32
    f16 = mybir.dt.float16
    f8 = mybir.dt.float8e4
    DR = mybir.MatmulPerfMode.DoubleRow

    nc = bacc.Bacc("TRN2", target_bir_lowering=False, debug=False,
                   num_devices=NCORES)
    # x shards, transposed: [in_features, m]
    x8t = nc.dram_tensor("x8t", [128, NP8, 2, M_CORE], f8,
                         kind="ExternalInput")
    x16t = nc.dram_tensor("x16t", [128, KT16, M_CORE], f16,
                          kind="ExternalInput")
    # pre-dequantized weights, transposed: [in_features, out_features]
    w8 = nc.dram_tensor("w8", [128, NP8, 2, OUT_F], f8,
                        kind="ExternalInput")
    w16 = nc.dram_tensor("w16", [KT16 * 128, OUT_F], f16,
                         kind="ExternalInput")

    # transposed output: [OUT_F, M_CORE]
    out = nc.dram_tensor("out", [OUT_F, M_CORE], f16, kind="ExternalOutput")

    with tile.TileContext(nc) as tc:
        with (
            tc.tile_pool(name="xres", bufs=1) as xres_pool,
            tc.tile_pool(name="wst", bufs=10) as wst_pool,
            tc.tile_pool(name="wdl", bufs=STEPS) as wdl_pool,
            tc.tile_pool(name="ost", bufs=12) as ost_pool,
            tc.tile_pool(name="psum", bufs=8, space="PSUM") as psum_pool,
        ):
            x8res = xres_pool.tile([128, NP8, 2, M_CORE], f8)
            x16res = xres_pool.tile([128, KT16, M_CORE], f16)

            warm_l = wst_pool.tile([128, 128], f16, name="warm_l", tag="warm")
            warm_r = xres_pool.tile([128, 512], f16, name="warm_r")
            nc.vector.memset(warm_l[:], 0.0)
            nc.vector.memset(warm_r[:], 0.0)
            warm_ps = psum_pool.tile([128, 512], f32, name="warm_ps",
                                     tag="ps")
            for _ in range(NWARM):
                nc.tensor.matmul(warm_ps[:], warm_l[:], warm_r[:],
                                 start=True, stop=True)

            wts = {}

            def produce(j):
                n, st = divmod(j, STEPS)
                o0 = n * 512
                pool = wdl_pool if n == NCH - 1 else wst_pool
                tg = "wdl" if pool is wdl_pool else "wst"
                if st < NP8:
                    a = st
                    if n == 0:
                        nc.sync.dma_start(x8res[:, a, :, :],
                                          x8t[:, a, :, :])
                    wt = pool.tile([128, 2, 512], f8,
                                   name=f"w8_{n}_{a}", tag=tg)
                    nc.gpsimd.dma_start(wt[:], w8[:, a, :, o0:o0 + 512])
                else:
                    kt = st - NP8
                    if n == 0:
                        nc.sync.dma_start(x16res[:, kt, :],
                                          x16t[:, kt, :])
                    wt = pool.tile([128, 512], f16,
                                   name=f"w16_{n}_{kt}", tag=tg)
                    nc.gpsimd.dma_start(
                        wt[:], w16[kt * 128:(kt + 1) * 128, o0:o0 + 512])
                wts[j] = wt

            psums = None

            def consume(j):
                nonlocal psums
                n, st = divmod(j, STEPS)
                last = n == NCH - 1
                mhs = (0,) if last else (0, 1)
                if st == 0:
                    psums = [psum_pool.tile([128, 512], f32,
                                            name=f"ps{n}_{o}_{mh}",
                                            tag="ps")
                             for o in range(4) for mh in mhs]
                wt = wts.pop(j) if not last else wts[j]
                for o in range(4):
                    for mh in mhs:
                        ps = psums[o * len(mhs) + mh]
                        if st < NP8:
                            nc.tensor.matmul(
                                ps[:],
                                wt[:, :, o * 128:(o + 1) * 128],
                                x8res[:, st, :, mh * 512:(mh + 1) * 512],
                                start=(st == 0),
                                stop=(st == STEPS - 1),
                                perf_mode=DR,
                            )
                        else:
                            nc.tensor.matmul(
                                ps[:],
                                wt[:, o * 128:(o + 1) * 128],
                                x16res[:, st - NP8,
                                       mh * 512:(mh + 1) * 512],
                                start=(st == 0),
                                stop=(st == STEPS - 1),
                            )
                if st == STEPS - 1 and not last:
                    o0 = n * 512
                    for o in range(4):
                        for mh in (0, 1):
                            ost = ost_pool.tile([128, 512], f16,
                                                name=f"ost{n}_{o}_{mh}",
                                                tag="ost")
                            # split evictions across ACT and DVE so the
                            # next chunk's first matmuls (which reuse
                            # these PSUM banks) unblock twice as fast
                            if mh == 0:
                                nc.scalar.copy(ost[:], psums[o * 2 + mh][:])
                            else:
                                nc.vector.tensor_copy(
                                    ost[:], psums[o * 2 + mh][:])
                            nc.gpsimd.dma_start(
                                out[o0 + o * 128:o0 + (o + 1) * 128,
                                    mh * 512:(mh + 1) * 512],
                                ost[:])

            for j in range(TOT + PREF):
                if j < TOT:
                    produce(j)
                jc = j - PREF
                if jc >= 0:
                    consume(jc)

            # Last chunk finale: phase A was mh=0 (m 0..511); phase B
            # redoes the step sweep for mh=1 on the resident weight tiles.
            n = NCH - 1
            o0 = n * 512
            psA = psums
            for half in (0, 1):
                if half == 1:
                    psB = [psum_pool.tile([128, 512], f32,
                                          name=f"ps{n}_{o}_1", tag="ps")
                           for o in range(4)]
                    for st in range(STEPS):
                        wt = wts[n * STEPS + st]
                        for o in range(4):
                            if st < NP8:
                                nc.tensor.matmul(
                                    psB[o][:],
                                    wt[:, :, o * 128:(o + 1) * 128],
                                    x8res[:, st, :, 512:1024],
                                    start=(st == 0),
                                    stop=(st == STEPS - 1),
                                    perf_mode=DR,
                                )
                            else:
                                nc.tensor.matmul(
                                    psB[o][:],
                                    wt[:, o * 128:(o + 1) * 128],
                                    x16res[:, st - NP8, 512:1024],
                                    start=(st == 0),
                                    stop=(st == STEPS - 1),
                                )
                ps = psA if half == 0 else psB
                for o in range(4):
                    ost = ost_pool.tile([128, 512], f16,
                                        name=f"ost{n}_{o}_{half}",
                                        tag="ost")
                    if o % 2 == 0:
                        nc.vector.tensor_copy(ost[:], ps[o][:])
                    else:
                        nc.scalar.copy(ost[:], ps[o][:])
                    if half == 0:
                        dma = nc.gpsimd
                    else:
                        dma = nc.sync if o % 2 == 0 else nc.scalar
                    dma.dma_start(
                        out[o0 + o * 128:o0 + (o + 1) * 128,
                            half * 512:(half + 1) * 512],
                        ost[:])

    nc.compile()
    return nc


def kernel(x: np.ndarray, ternary: np.ndarray, scales: np.ndarray,
           _trace: bool = False):
    import ml_dtypes
    from concourse.bass_utils import run_bass_kernel_spmd

    nc = _build()
    f8 = ml_dtypes.float8_e4m3

    x = np.asarray(x)
    ternary = np.asarray(ternary)
    scales = np.asarray(scales)

    # Dequantize on the host: W[o, i] = ternary[o, i] * scales[o, i//128]
    w = (ternary.astype(np.float32).reshape(-1, 128)
         * np.asarray(scales, dtype=np.float32)[:, None]).reshape(OUT_F, IN_F)
    wT = np.ascontiguousarray(w.T)  # [in, out]
    K8 = 2 * NP8 * 128  # in_features handled in fp8
    # [128, NP8, 2, OUT_F]: w8h[k, a, j, o] = Q8(wT[(2a+j)*128 + k, o])
    w8h = np.ascontiguousarray(
        wT[:K8].reshape(NP8, 2, 128, OUT_F).transpose(2, 0, 1, 3)
    ).astype(f8)
    w16h = wT[K8:].astype(np.float16)

    xf = x.reshape(M_TOT, IN_F)
    in_maps = []
    for c in range(NCORES):
        xcT = xf[c * M_CORE:(c + 1) * M_CORE, :].T  # [in, m]
        x8h = np.ascontiguousarray(
            xcT[:K8].reshape(NP8, 2, 128, M_CORE).transpose(2, 0, 1, 3)
        ).astype(f8)
        x16h = np.ascontiguousarray(
            xcT[K8:].reshape(KT16, 128, M_CORE).transpose(1, 0, 2)
        ).astype(np.float16)
        in_maps.append({"x8t": x8h, "x16t": x16h, "w8": w8h, "w16": w16h})

    res = run_bass_kernel_spmd(nc, in_maps, list(range(NCORES)),
                               trace=_trace)
    # out is [OUT_F, M_CORE] per core; untranspose on the host
    outs = [res.results[c]["out"].T for c in range(NCORES)]
    full = np.concatenate(outs, axis=0).astype(np.float32).reshape(B, S, OUT_F)
    if _trace:
        kernel.last_results = res
    return full


kernel.last_results = None


# revision 6
# speedup vs baseline: 1.0283x; 1.0050x over previous
"""ECPGLinear Bass kernel: hybrid fp8/fp16 + one-level Strassen on the fp16 part.

out = x @ W.T (W = ternary * group scales), 8192x4096x4096, data-parallel
over tokens across 8 cores (1024 rows each).

Precision split (unchanged from the direct hybrid): in_features 0..1023
in fp8e4m3 DoubleRow pairs (2x PE rate), 1024..4095 in fp16. Measured
rel err 1.898e-2 vs the 2e-2 budget.

The fp16 part (W16 [4096o x 3072i] @ X16 [3072i x 1024m]) runs one level
of Strassen: o->2x2048, i->2x1536, m->2x512. Host precomputes the 7
stationary combos (A*) and 7 moving combos (B*) in fp16; the device does
7 block-products per o-subtile instead of 8 (84 fp16 matmuls vs 96), a
12.5% PE saving. M-products stay in PSUM f32; quadrant combines run on
DVE/Pool with f32 SBUF intermediates, so the extra numeric error is
negligible (measured 1.8980e-2 total).

fp8 partials F(o-row, m-half): F11 accumulates into M7's PSUM group and
F22 into M6's (M7/M6 appear only in C11/C22 respectively); F12/F21 use a
rotating spare bank and are read directly by the combines. Peak PSUM use
is 6 of 8 banks; per o-subtile there are 10 combine passes, overlapped
with the next products' matmuls.

Per o-subtile s (16 total; o-rows s*128 and 2048+s*128):
  M1,M4,M5,(M7+F11),C11, M2,F12, M3,(M6+F22), C12, F21, C21, C22
  C11 = M1+M4-M5+M7F11      -> out[s*128,    0:512]
  C12 = M3+M5+F12           -> out[s*128,    512:1024]
  C21 = M2+M4+F21           -> out[2048+s*128, 0:512]
  C22 = M1-M2+M3+M6F22      -> out[2048+s*128, 512:1024]
"""
import functools
import numpy as np

OUT_F = 4096
IN_F = 4096
B, S = 4, 2048
M_TOT = B * S
NCORES = 8
M_CORE = M_TOT // NCORES
NP8 = 4                  # fp8 DoubleRow k-pairs (i 0..1023)
KT16 = 12                # fp16 k-tiles per Strassen i-half (1536/128)
NSUB = 16                # o-subtiles (2048/128)
NWARM = 13
# Strassen products in device emission order, with their A/B combo index
# (1-based classic numbering): C11 inputs first so M7's bank frees early.
PROD_ORDER = [1, 4, 5, 7, 2, 3, 6]


@functools.lru_cache(maxsize=1)
def _build():
    from concourse import bacc
    import concourse.mybir as mybir
    import concourse.tile as tile

    from concourse.alu_op_type import AluOpType

    f32 = mybir.dt.float32
    f16 = mybir.dt.float16
    f8 = mybir.dt.float8e4
    DR = mybir.MatmulPerfMode.DoubleRow
    SUB = AluOpType.subtract

    nc = bacc.Bacc("TRN2", target_bir_lowering=False, debug=False,
                   num_devices=NCORES)
    x8t = nc.dram_tensor("x8t", [128, NP8, 2, M_CORE], f8,
                         kind="ExternalInput")
    # B* combos: [7, 128, 12, 512] (combo, partition, kt, m-half col)
    xbt = nc.dram_tensor("xbt", [7, 128, KT16, 512], f16,
                         kind="ExternalInput")
    w8 = nc.dram_tensor("w8", [128, NP8, 2, OUT_F], f8,
                        kind="ExternalInput")
    # A* combos: [7, 128, 12, 2048]
    wat = nc.dram_tensor("wat", [7, 128, KT16, 2048], f16,
                         kind="ExternalInput")
    out = nc.dram_tensor("out", [OUT_F, M_CORE], f16, kind="ExternalOutput")

    with tile.TileContext(nc) as tc:
        with (
            tc.tile_pool(name="xres", bufs=1) as xres_pool,
            tc.tile_pool(name="wa", bufs=16) as wa_pool,
            tc.tile_pool(name="w8p", bufs=5) as w8_pool,
            tc.tile_pool(name="tmp", bufs=10) as tmp_pool,
            tc.tile_pool(name="ost", bufs=8) as ost_pool,
            tc.tile_pool(name="psum", bufs=8, space="PSUM") as psum_pool,
        ):
            x8res = xres_pool.tile([128, NP8, 2, M_CORE], f8)
            bres = {k: xres_pool.tile([128, KT16, 512], f16,
                                      name=f"bres{k}")
                    for k in PROD_ORDER}

            warm_l = wa_pool.tile([128, 128], f16, name="warm_l", tag="warm")
            warm_r = xres_pool.tile([128, 512], f16, name="warm_r")
            nc.vector.memset(warm_l[:], 0.0)
            nc.vector.memset(warm_r[:], 0.0)
            warm_ps = psum_pool.tile([128, 512], f32, name="warm_ps",
                                     tag="ps")
            for _ in range(NWARM):
                nc.tensor.matmul(warm_ps[:], warm_l[:], warm_r[:],
                                 start=True, stop=True)

            # resident loads, consumption-ordered and split into
            # kt-thirds across three queues so osub 0/1 are fed at
            # aggregate HBM rate: PE consumes B* combos within ~21us of
            # start, which one queue cannot sustain.
            nc.sync.dma_start(x8res[:], x8t[:, :, :, :])
            for k in PROD_ORDER:
                nc.sync.dma_start(bres[k][:], xbt[k - 1, :, :, :])

            was = {}
            w8s = {}

            def produce(s):
                # stream osub s's stationary tiles: 7 A-combo tiles + 2
                # fp8 weight tiles
                for i, k in enumerate(PROD_ORDER):
                    wt = wa_pool.tile([128, KT16, 128], f16,
                                      name=f"wa{s}_{k}", tag="wa")
                    nc.gpsimd.dma_start(
                        wt[:], wat[k - 1, :, :, s * 128:(s + 1) * 128])
                    was[(s, k)] = wt
                for row, orow in ((1, s * 128), (2, 2048 + s * 128)):
                    w8tile = w8_pool.tile([128, NP8, 2, 128], f8,
                                          name=f"w8_{s}_{row}", tag="w8")
                    nc.gpsimd.dma_start(
                        w8tile[:], w8[:, :, :, orow:orow + 128])
                    w8s[(s, row)] = w8tile

            def matmul_M(ps, s, k, with_F=None):
                """12 fp16 matmuls for product k of osub s; optionally
                the accumulation group opens with 4 fp8 DR matmuls
                (with_F = (w8tile, mh)) — DR-first matches the verified
                mixed-group pattern of the direct hybrid kernel."""
                if with_F is not None:
                    w8tile, mh = with_F
                    for a in range(NP8):
                        nc.tensor.matmul(
                            ps[:], w8tile[:, a, :, :],
                            x8res[:, a, :, mh * 512:(mh + 1) * 512],
                            start=(a == 0), stop=False,
                            perf_mode=DR,
                        )
                wt = was.pop((s, k))
                for kt in range(KT16):
                    nc.tensor.matmul(
                        ps[:], wt[:, kt, :], bres[k][:, kt, :],
                        start=(kt == 0 and with_F is None),
                        stop=(kt == KT16 - 1),
                    )

            def matmul_F(ps, w8tile, mh):
                for a in range(NP8):
                    nc.tensor.matmul(
                        ps[:], w8tile[:, a, :, :],
                        x8res[:, a, :, mh * 512:(mh + 1) * 512],
                        start=(a == 0), stop=(a == NP8 - 1),
                        perf_mode=DR,
                    )

            def store(s, quad, ost):
                o0 = s * 128 if quad[0] == '1' else 2048 + s * 128
                m0 = 0 if quad[1] == '1' else 512
                dma = nc.scalar if quad in ('11', '22') else nc.sync
                dma.dma_start(out[o0:o0 + 128, m0:m0 + 512], ost[:])

            PREF = 2
            for s in range(PREF):
                produce(s)
            for s in range(NSUB):
                if s + PREF < NSUB:
                    produce(s + PREF)
                w8a = w8s.pop((s, 1))
                w8b = w8s.pop((s, 2))
                M = {k: psum_pool.tile([128, 512], f32,
                                       name=f"M{s}_{k}", tag="ps")
                     for k in PROD_ORDER}
                matmul_M(M[1], s, 1)
                # e1 frees M1's bank early; every DVE op below keeps its
                # single PSUM operand in position 0 (HW port rule)
                e1 = tmp_pool.tile([128, 512], f32, name=f"e1_{s}",
                                   tag="tmp")
                nc.scalar.copy(e1[:], M[1][:])
                matmul_M(M[4], s, 4)
                u1 = tmp_pool.tile([128, 512], f32, name=f"u1_{s}",
                                   tag="tmp")
                nc.vector.tensor_add(u1[:], M[4][:], e1[:])
                matmul_M(M[5], s, 5)
                e5 = tmp_pool.tile([128, 512], f32, name=f"e5_{s}",
                                   tag="tmp")
                nc.scalar.copy(e5[:], M[5][:])
                matmul_M(M[7], s, 7, with_F=(w8a, 0))     # M7 += F11
                u2 = tmp_pool.tile([128, 512], f32, name=f"u2_{s}",
                                   tag="tmp")
                nc.vector.tensor_tensor(u2[:], M[7][:], e5[:], op=SUB)
                ost11 = ost_pool.tile([128, 512], f16, name=f"o11_{s}",
                                      tag="ost")
                nc.gpsimd.tensor_add(ost11[:], u1[:], u2[:])
                store(s, '11', ost11)

                matmul_M(M[2], s, 2)
                e2 = tmp_pool.tile([128, 512], f32, name=f"e2_{s}",
                                   tag="tmp")
                nc.scalar.copy(e2[:], M[2][:])
                F12 = psum_pool.tile([128, 512], f32, name=f"F12_{s}",
                                     tag="ps")
                matmul_F(F12, w8a, 1)
                matmul_M(M[3], s, 3)
                # C12 = M3+M5+F12
                p = tmp_pool.tile([128, 512], f32, name=f"p_{s}",
                                  tag="tmp")
                nc.vector.tensor_add(p[:], M[3][:], e5[:])
                ost12 = ost_pool.tile([128, 512], f16, name=f"o12_{s}",
                                      tag="ost")
                nc.vector.tensor_add(ost12[:], F12[:], p[:])
                store(s, '12', ost12)

                # C21 = M2+M4+F21
                q = tmp_pool.tile([128, 512], f32, name=f"q_{s}",
                                  tag="tmp")
                nc.vector.tensor_add(q[:], M[4][:], e2[:])
                matmul_M(M[6], s, 6, with_F=(w8b, 1))     # M6 += F22
                F21 = psum_pool.tile([128, 512], f32, name=f"F21_{s}",
                                     tag="ps")
                matmul_F(F21, w8b, 0)
                ost21 = ost_pool.tile([128, 512], f16, name=f"o21_{s}",
                                      tag="ost")
                nc.vector.tensor_add(ost21[:], F21[:], q[:])
                store(s, '21', ost21)

                # C22 = M1-M2+M3+M6F22
                v = tmp_pool.tile([128, 512], f32, name=f"v_{s}",
                                  tag="tmp")
                nc.gpsimd.tensor_tensor(v[:], e1[:], e2[:], op=SUB)
                z = tmp_pool.tile([128, 512], f32, name=f"z_{s}",
                                  tag="tmp")
                nc.vector.tensor_add(z[:], M[3][:], v[:])
                ost22 = ost_pool.tile([128, 512], f16, name=f"o22_{s}",
                                      tag="ost")
                nc.vector.tensor_add(ost22[:], M[6][:], z[:])
                store(s, '22', ost22)

    nc.compile()
    return nc


def kernel(x: np.ndarray, ternary: np.ndarray, scales: np.ndarray,
           _trace: bool = False):
    import ml_dtypes
    from concourse.bass_utils import run_bass_kernel_spmd

    nc = _build()
    f8 = ml_dtypes.float8_e4m3

    x = np.asarray(x)
    ternary = np.asarray(ternary)
    scales = np.asarray(scales)

    w = (ternary.astype(np.float32).reshape(-1, 128)
         * np.asarray(scales, dtype=np.float32)[:, None]).reshape(OUT_F, IN_F)
    wT = np.ascontiguousarray(w.T)  # [in, out]
    K8 = 2 * NP8 * 128
    w8h = np.ascontiguousarray(
        wT[:K8].reshape(NP8, 2, 128, OUT_F).transpose(2, 0, 1, 3)
    ).astype(f8)

    # A* combos from W16 [4096o, 3072i]
    W16 = w[:, K8:]
    A11, A12 = W16[:2048, :1536], W16[:2048, 1536:]
    A21, A22 = W16[2048:, :1536], W16[2048:, 1536:]
    Acombos = {1: A11 + A22, 2: A21 + A22, 3: A11, 4: A22,
               5: A11 + A12, 6: A21 - A11, 7: A12 - A22}
    wah = np.empty((7, 128, KT16, 2048), dtype=np.float16)
    for k, Ak in Acombos.items():
        # [2048o, 1536i] -> [i, o] -> [128p, 12kt, 2048o]
        wah[k - 1] = (Ak.T.reshape(KT16, 128, 2048).transpose(1, 0, 2)
                      .astype(np.float16))

    xf = x.reshape(M_TOT, IN_F)
    in_maps = []
    for c in range(NCORES):
        xcT = xf[c * M_CORE:(c + 1) * M_CORE, :].T  # [in, m]
        x8h = np.ascontiguousarray(
            xcT[:K8].reshape(NP8, 2, 128, M_CORE).transpose(2, 0, 1, 3)
        ).astype(f8)
        X16 = xcT[K8:]  # [3072, 1024]
        B11, B12 = X16[:1536, :512], X16[:1536, 512:]
        B21, B22 = X16[1536:, :512], X16[1536:, 512:]
        Bcombos = {1: B11 + B22, 2: B11, 3: B12 - B22, 4: B21 - B11,
                   5: B22, 6: B11 + B12, 7: B21 + B22}
        xbh = np.empty((7, 128, KT16, 512), dtype=np.float16)
        for k, Bk in Bcombos.items():
            xbh[k - 1] = (Bk.reshape(KT16, 128, 512).transpose(1, 0, 2)
                          .astype(np.float16))
        in_maps.append({"x8t": x8h, "xbt": xbh, "w8": w8h, "wat": wah})

    res = run_bass_kernel_spmd(nc, in_maps, list(range(NCORES)),
                               trace=_trace)
    outs = [res.results[c]["out"].T for c in range(NCORES)]
    full = np.concatenate(outs, axis=0).astype(np.float32).reshape(B, S, OUT_F)
    if _trace:
        kernel.last_results = res
    return full


kernel.last_results = None


# revision 7
# speedup vs baseline: 1.0310x; 1.0027x over previous
"""ECPGLinear Bass kernel: hybrid fp8/fp16 + one-level Strassen on the fp16 part.

out = x @ W.T (W = ternary * group scales), 8192x4096x4096, data-parallel
over tokens across 8 cores (1024 rows each).

Precision split (unchanged from the direct hybrid): in_features 0..1023
in fp8e4m3 DoubleRow pairs (2x PE rate), 1024..4095 in fp16. Measured
rel err 1.898e-2 vs the 2e-2 budget.

The fp16 part (W16 [4096o x 3072i] @ X16 [3072i x 1024m]) runs one level
of Strassen: o->2x2048, i->2x1536, m->2x512. Host precomputes the 7
stationary combos (A*) and 7 moving combos (B*) in fp16; the device does
7 block-products per o-subtile instead of 8 (84 fp16 matmuls vs 96), a
12.5% PE saving. M-products stay in PSUM f32; quadrant combines run on
DVE/Pool with f32 SBUF intermediates, so the extra numeric error is
negligible (measured 1.8980e-2 total).

fp8 partials F(o-row, m-half): F11 accumulates into M7's PSUM group and
F22 into M6's (M7/M6 appear only in C11/C22 respectively); F12/F21 use a
rotating spare bank and are read directly by the combines. Peak PSUM use
is 6 of 8 banks; per o-subtile there are 10 combine passes, overlapped
with the next products' matmuls.

Per o-subtile s (16 total; o-rows s*128 and 2048+s*128):
  M1,M4,M5,(M7+F11),C11, M2,F12, M3,(M6+F22), C12, F21, C21, C22
  C11 = M1+M4-M5+M7F11      -> out[s*128,    0:512]
  C12 = M3+M5+F12           -> out[s*128,    512:1024]
  C21 = M2+M4+F21           -> out[2048+s*128, 0:512]
  C22 = M1-M2+M3+M6F22      -> out[2048+s*128, 512:1024]
"""
import functools
import numpy as np

OUT_F = 4096
IN_F = 4096
B, S = 4, 2048
M_TOT = B * S
NCORES = 8
M_CORE = M_TOT // NCORES
NP8 = 4                  # fp8 DoubleRow k-pairs (i 0..1023)
KT16 = 12                # fp16 k-tiles per Strassen i-half (1536/128)
NSUB = 16                # o-subtiles (2048/128)
NWARM = 13
# Strassen products in device emission order, with their A/B combo index
# (1-based classic numbering): C11 inputs first so M7's bank frees early.
PROD_ORDER = [1, 4, 5, 7, 2, 3, 6]


@functools.lru_cache(maxsize=1)
def _build():
    from concourse import bacc
    import concourse.mybir as mybir
    import concourse.tile as tile

    from concourse.alu_op_type import AluOpType

    f32 = mybir.dt.float32
    f16 = mybir.dt.float16
    f8 = mybir.dt.float8e4
    DR = mybir.MatmulPerfMode.DoubleRow
    SUB = AluOpType.subtract

    nc = bacc.Bacc("TRN2", target_bir_lowering=False, debug=False,
                   num_devices=NCORES)
    x8t = nc.dram_tensor("x8t", [128, NP8, 2, M_CORE], f8,
                         kind="ExternalInput")
    # B* combos: [7, 128, 12, 512] (combo, partition, kt, m-half col)
    xbt = nc.dram_tensor("xbt", [7, 128, KT16, 512], f16,
                         kind="ExternalInput")
    w8 = nc.dram_tensor("w8", [128, NP8, 2, OUT_F], f8,
                        kind="ExternalInput")
    # A* combos: [7, 128, 12, 2048]
    wat = nc.dram_tensor("wat", [7, 128, KT16, 2048], f16,
                         kind="ExternalInput")
    out = nc.dram_tensor("out", [OUT_F, M_CORE], f16, kind="ExternalOutput")

    with tile.TileContext(nc) as tc:
        with (
            tc.tile_pool(name="xres", bufs=1) as xres_pool,
            tc.tile_pool(name="wa", bufs=16) as wa_pool,
            tc.tile_pool(name="w8p", bufs=5) as w8_pool,
            tc.tile_pool(name="tmp", bufs=10) as tmp_pool,
            tc.tile_pool(name="ost", bufs=8) as ost_pool,
            tc.tile_pool(name="psum", bufs=8, space="PSUM") as psum_pool,
        ):
            x8res = xres_pool.tile([128, NP8, 2, M_CORE], f8)
            bres = {k: xres_pool.tile([128, KT16, 512], f16,
                                      name=f"bres{k}")
                    for k in PROD_ORDER}

            warm_l = wa_pool.tile([128, 128], f16, name="warm_l", tag="warm")
            warm_r = xres_pool.tile([128, 512], f16, name="warm_r")
            nc.vector.memset(warm_l[:], 0.0)
            nc.vector.memset(warm_r[:], 0.0)
            warm_ps = psum_pool.tile([128, 512], f32, name="warm_ps",
                                     tag="ps")
            for _ in range(NWARM):
                nc.tensor.matmul(warm_ps[:], warm_l[:], warm_r[:],
                                 start=True, stop=True)

            # resident loads on the sync queue, consumption-ordered
            # (x8 feeds the first F at ~11us; b1 gates the first
            # product). The startup is HBM/queue-bound: ~12MB must land
            # during the first 2-3 o-subtiles.
            nc.sync.dma_start(x8res[:], x8t[:, :, :, :])
            for k in PROD_ORDER:
                nc.sync.dma_start(bres[k][:], xbt[k - 1, :, :, :])

            was = {}
            w8s = {}

            def produce(s):
                # stream osub s's stationary tiles: 7 A-combo tiles + 2
                # fp8 weight tiles
                for i, k in enumerate(PROD_ORDER):
                    wt = wa_pool.tile([128, KT16, 128], f16,
                                      name=f"wa{s}_{k}", tag="wa")
                    nc.gpsimd.dma_start(
                        wt[:], wat[k - 1, :, :, s * 128:(s + 1) * 128])
                    was[(s, k)] = wt
                for row, orow in ((1, s * 128), (2, 2048 + s * 128)):
                    w8tile = w8_pool.tile([128, NP8, 2, 128], f8,
                                          name=f"w8_{s}_{row}", tag="w8")
                    nc.gpsimd.dma_start(
                        w8tile[:], w8[:, :, :, orow:orow + 128])
                    w8s[(s, row)] = w8tile

            def matmul_M(ps, s, k, with_F=None):
                """12 fp16 matmuls for product k of osub s; optionally
                the accumulation group opens with 4 fp8 DR matmuls
                (with_F = (w8tile, mh)) — DR-first matches the verified
                mixed-group pattern of the direct hybrid kernel."""
                if with_F is not None:
                    w8tile, mh = with_F
                    for a in range(NP8):
                        nc.tensor.matmul(
                            ps[:], w8tile[:, a, :, :],
                            x8res[:, a, :, mh * 512:(mh + 1) * 512],
                            start=(a == 0), stop=False,
                            perf_mode=DR,
                        )
                wt = was.pop((s, k))
                for kt in range(KT16):
                    nc.tensor.matmul(
                        ps[:], wt[:, kt, :], bres[k][:, kt, :],
                        start=(kt == 0 and with_F is None),
                        stop=(kt == KT16 - 1),
                    )

            def matmul_F(ps, w8tile, mh):
                for a in range(NP8):
                    nc.tensor.matmul(
                        ps[:], w8tile[:, a, :, :],
                        x8res[:, a, :, mh * 512:(mh + 1) * 512],
                        start=(a == 0), stop=(a == NP8 - 1),
                        perf_mode=DR,
                    )

            def store(s, quad, ost):
                o0 = s * 128 if quad[0] == '1' else 2048 + s * 128
                m0 = 0 if quad[1] == '1' else 512
                dma = nc.scalar if quad in ('11', '22') else nc.sync
                dma.dma_start(out[o0:o0 + 128, m0:m0 + 512], ost[:])

            PREF = 1
            for s in range(PREF):
                produce(s)
            for s in range(NSUB):
                if s + PREF < NSUB:
                    produce(s + PREF)
                w8a = w8s.pop((s, 1))
                w8b = w8s.pop((s, 2))
                M = {k: psum_pool.tile([128, 512], f32,
                                       name=f"M{s}_{k}", tag="ps")
                     for k in PROD_ORDER}
                matmul_M(M[1], s, 1)
                # e1 frees M1's bank early; every DVE op below keeps its
                # single PSUM operand in position 0 (HW port rule)
                e1 = tmp_pool.tile([128, 512], f32, name=f"e1_{s}",
                                   tag="tmp")
                nc.scalar.copy(e1[:], M[1][:])
                matmul_M(M[4], s, 4)
                u1 = tmp_pool.tile([128, 512], f32, name=f"u1_{s}",
                                   tag="tmp")
                nc.vector.tensor_add(u1[:], M[4][:], e1[:])
                matmul_M(M[5], s, 5)
                e5 = tmp_pool.tile([128, 512], f32, name=f"e5_{s}",
                                   tag="tmp")
                nc.scalar.copy(e5[:], M[5][:])
                matmul_M(M[7], s, 7, with_F=(w8a, 0))     # M7 += F11
                u2 = tmp_pool.tile([128, 512], f32, name=f"u2_{s}",
                                   tag="tmp")
                nc.vector.tensor_tensor(u2[:], M[7][:], e5[:], op=SUB)
                ost11 = ost_pool.tile([128, 512], f16, name=f"o11_{s}",
                                      tag="ost")
                nc.gpsimd.tensor_add(ost11[:], u1[:], u2[:])
                store(s, '11', ost11)

                matmul_M(M[2], s, 2)
                e2 = tmp_pool.tile([128, 512], f32, name=f"e2_{s}",
                                   tag="tmp")
                nc.scalar.copy(e2[:], M[2][:])
                F12 = psum_pool.tile([128, 512], f32, name=f"F12_{s}",
                                     tag="ps")
                matmul_F(F12, w8a, 1)
                matmul_M(M[3], s, 3)
                # C12 = M3+M5+F12
                p = tmp_pool.tile([128, 512], f32, name=f"p_{s}",
                                  tag="tmp")
                nc.vector.tensor_add(p[:], M[3][:], e5[:])
                ost12 = ost_pool.tile([128, 512], f16, name=f"o12_{s}",
                                      tag="ost")
                nc.vector.tensor_add(ost12[:], F12[:], p[:])
                store(s, '12', ost12)

                # C21 = M2+M4+F21
                q = tmp_pool.tile([128, 512], f32, name=f"q_{s}",
                                  tag="tmp")
                nc.vector.tensor_add(q[:], M[4][:], e2[:])
                matmul_M(M[6], s, 6, with_F=(w8b, 1))     # M6 += F22
                F21 = psum_pool.tile([128, 512], f32, name=f"F21_{s}",
                                     tag="ps")
                matmul_F(F21, w8b, 0)
                ost21 = ost_pool.tile([128, 512], f16, name=f"o21_{s}",
                                      tag="ost")
                nc.vector.tensor_add(ost21[:], F21[:], q[:])
                store(s, '21', ost21)

                # C22 = M1-M2+M3+M6F22
                v = tmp_pool.tile([128, 512], f32, name=f"v_{s}",
                                  tag="tmp")
                nc.gpsimd.tensor_tensor(v[:], e1[:], e2[:], op=SUB)
                z = tmp_pool.tile([128, 512], f32, name=f"z_{s}",
                                  tag="tmp")
                nc.vector.tensor_add(z[:], M[3][:], v[:])
                ost22 = ost_pool.tile([128, 512], f16, name=f"o22_{s}",
                                      tag="ost")
                nc.vector.tensor_add(ost22[:], M[6][:], z[:])
                store(s, '22', ost22)

    nc.compile()
    return nc


def kernel(x: np.ndarray, ternary: np.ndarray, scales: np.ndarray,
           _trace: bool = False):
    import ml_dtypes
    from concourse.bass_utils import run_bass_kernel_spmd

    nc = _build()
    f8 = ml_dtypes.float8_e4m3

    x = np.asarray(x)
    ternary = np.asarray(ternary)
    scales = np.asarray(scales)

    w = (ternary.astype(np.float32).reshape(-1, 128)
         * np.asarray(scales, dtype=np.float32)[:, None]).reshape(OUT_F, IN_F)
    wT = np.ascontiguousarray(w.T)  # [in, out]
    K8 = 2 * NP8 * 128
    w8h = np.ascontiguousarray(
        wT[:K8].reshape(NP8, 2, 128, OUT_F).transpose(2, 0, 1, 3)
    ).astype(f8)

    # A* combos from W16 [4096o, 3072i]
    W16 = w[:, K8:]
    A11, A12 = W16[:2048, :1536], W16[:2048, 1536:]
    A21, A22 = W16[2048:, :1536], W16[2048:, 1536:]
    Acombos = {1: A11 + A22, 2: A21 + A22, 3: A11, 4: A22,
               5: A11 + A12, 6: A21 - A11, 7: A12 - A22}
    wah = np.empty((7, 128, KT16, 2048), dtype=np.float16)
    for k, Ak in Acombos.items():
        # [2048o, 1536i] -> [i, o] -> [128p, 12kt, 2048o]
        wah[k - 1] = (Ak.T.reshape(KT16, 128, 2048).transpose(1, 0, 2)
                      .astype(np.float16))

    xf = x.reshape(M_TOT, IN_F)
    in_maps = []
    for c in range(NCORES):
        xcT = xf[c * M_CORE:(c + 1) * M_CORE, :].T  # [in, m]
        x8h = np.ascontiguousarray(
            xcT[:K8].reshape(NP8, 2, 128, M_CORE).transpose(2, 0, 1, 3)
        ).astype(f8)
        X16 = xcT[K8:]  # [3072, 1024]
        B11, B12 = X16[:1536, :512], X16[:1536, 512:]
        B21, B22 = X16[1536:, :512], X16[1536:, 512:]
        Bcombos = {1: B11 + B22, 2: B11, 3: B12 - B22, 4: B21 - B11,
                   5: B22, 6: B11 + B12, 7: B21 + B22}
        xbh = np.empty((7, 128, KT16, 512), dtype=np.float16)
        for k, Bk in Bcombos.items():
            xbh[k - 1] = (Bk.reshape(KT16, 128, 512).transpose(1, 0, 2)
                          .astype(np.float16))
        in_maps.append({"x8t": x8h, "xbt": xbh, "w8": w8h, "wat": wah})

    res = run_bass_kernel_spmd(nc, in_maps, list(range(NCORES)),
                               trace=_trace)
    outs = [res.results[c]["out"].T for c in range(NCORES)]
    full = np.concatenate(outs, axis=0).astype(np.float32).reshape(B, S, OUT_F)
    if _trace:
        kernel.last_results = res
    return full


kernel.last_results = None
